# revision 11
# baseline (speedup 1.0000x reference)
import sys
import numpy as np

sys.path.insert(0, "/opt/trn_rl_repo")

B, T = 256, 512
DIM_TAG, DIM_COM, H = 194, 49, 32
N_CORES = 8
BC = B // N_CORES  # 32 batch rows per core
KA, KB = 66, 128   # feature split of 194 = 66 + 128

_PERM128 = None


def _perm128():
    # reorder gate rows [i,f,g,o] (PyTorch) -> [i,f,o,g] so sigmoid rows are 0:96
    global _PERM128
    if _PERM128 is None:
        _PERM128 = np.concatenate([np.arange(0, 64), np.arange(96, 128), np.arange(64, 96)])
    return _PERM128


def _build_nc(debug=False):
    import concourse.bass as bass
    import concourse.mybir as mybir
    from concourse.tile import TileContext

    f32 = mybir.dt.float32
    f16 = mybir.dt.float16
    AF = mybir.ActivationFunctionType

    nc = bass.Bass()
    DT = nc.dram_tensor

    x = DT("x", [BC, T, DIM_TAG], f16, kind="ExternalInput")
    xcf = DT("xcf", [DIM_COM, BC], f32, kind="ExternalInput")
    xcl = DT("xcl", [DIM_COM, BC], f32, kind="ExternalInput")
    idF = DT("idF", [128, 128], f16, kind="ExternalInput")
    idS = DT("idS", [32, 32], f32, kind="ExternalInput")
    mask = DT("mask", [128, 128], f16, kind="ExternalInput")
    # L0 input weights (pre_w folded), K split 0:66 / 66:194
    w0 = {}
    for d in ("f", "b"):
        w0["a" + d] = DT("w0a" + d, [KA, 128], f16, kind="ExternalInput")
        w0["b" + d] = DT("w0b" + d, [KB, 128], f16, kind="ExternalInput")
        w0["h" + d] = DT("whh0" + d, [33, 128], f16, kind="ExternalInput")
    w1 = {}
    for d in ("f", "b"):
        w1["f" + d] = DT("w1f" + d, [32, 128], f16, kind="ExternalInput")
        w1["b" + d] = DT("w1b" + d, [32, 128], f16, kind="ExternalInput")
        w1["h" + d] = DT("whh1" + d, [33, 128], f16, kind="ExternalInput")
    warf = DT("warf", [32, 16], f16, kind="ExternalInput")
    warb = DT("warb", [32, 16], f16, kind="ExternalInput")
    whha = DT("whha", [5, 16], f16, kind="ExternalInput")
    adh = DT("adh", [32, 4], f16, kind="ExternalInput")
    adc = DT("adc", [32, 4], f16, kind="ExternalInput")
    adhb = DT("adhb", [1, 4], f32, kind="ExternalInput")
    adcb = DT("adcb", [1, 4], f32, kind="ExternalInput")
    h0w = [DT(f"h0w{k}", [DIM_COM, 32], f32, kind="ExternalInput") for k in range(4)]
    c0w = [DT(f"c0w{k}", [DIM_COM, 32], f32, kind="ExternalInput") for k in range(4)]
    h0b = DT("h0b", [32, 4], f32, kind="ExternalInput")
    c0b = DT("c0b", [32, 4], f32, kind="ExternalInput")
    p1wA = DT("p1wA", [128, 128], f32, kind="ExternalInput")
    p1wB = DT("p1wB", [128, 128], f32, kind="ExternalInput")
    p1wC = DT("p1wC", [98, 128], f32, kind="ExternalInput")
    p1b = DT("p1b", [128, 1], f32, kind="ExternalInput")
    p2w = DT("p2w", [128, 128], f32, kind="ExternalInput")
    p2b = DT("p2b", [128, 1], f32, kind="ExternalInput")
    p3w = DT("p3w", [128, 16], f32, kind="ExternalInput")
    p3b = DT("p3b", [16, 1], f32, kind="ExternalInput")
    y = DT("y", [16, BC], f32, kind="ExternalOutput")
    dbg = {}
    if debug:
        dbg["x1S"] = DT("dbg_x1S", [33, T, 64], f16, kind="ExternalOutput")
        dbg["h1S"] = DT("dbg_h1S", [33, T, 64], f16, kind="ExternalOutput")
        dbg["hS"] = DT("dbg_hS", [5, T, BC], f16, kind="ExternalOutput")
        dbg["poolT"] = DT("dbg_poolT", [128, 65], f32, kind="ExternalOutput")

    with TileContext(nc) as tc:
        with (
            tc.tile_pool(name="const", bufs=1) as cp,
            tc.tile_pool(name="big", bufs=1) as bigp,
            tc.tile_pool(name="work", bufs=2) as wp,
            tc.tile_pool(name="psG", bufs=2, space="PSUM") as psG,
            tc.tile_pool(name="psC", bufs=1, space="PSUM") as psC,
            tc.tile_pool(name="psT", bufs=2, space="PSUM") as psT,
            tc.tile_pool(name="psO", bufs=1, space="PSUM") as psO,
        ):
            def ld(drt, shape, dt, tag):
                t = cp.tile(shape, dt, tag=tag, name="ld_" + tag)
                nc.sync.dma_start(out=t, in_=drt[:, :] if len(shape) == 2 else drt)
                return t

            # ---- constants to SBUF ----
            idF_s = ld(idF, [128, 128], f16, "idF")
            idS_s = ld(idS, [32, 32], f32, "idS")
            mask_s = ld(mask, [128, 128], f16, "mask")
            xcf_s = ld(xcf, [DIM_COM, BC], f32, "xcf")
            xcl_s = ld(xcl, [DIM_COM, BC], f32, "xcl")
            w0s = {k: ld(v, list(v.shape), f16, "w0" + k) for k, v in w0.items()}
            w1s = {k: ld(v, list(v.shape), f16, "w1" + k) for k, v in w1.items()}
            warf_s = ld(warf, [32, 16], f16, "warf")
            warb_s = ld(warb, [32, 16], f16, "warb")
            whha_s = ld(whha, [5, 16], f16, "whha")
            adh_s = ld(adh, [32, 4], f16, "adh")
            adc_s = ld(adc, [32, 4], f16, "adc")
            adhb_s = ld(adhb, [1, 4], f32, "adhb")
            adcb_s = ld(adcb, [1, 4], f32, "adcb")
            h0w_s = [ld(h0w[k], [DIM_COM, 32], f32, f"h0w{k}") for k in range(4)]
            c0w_s = [ld(c0w[k], [DIM_COM, 32], f32, f"c0w{k}") for k in range(4)]
            h0b_s = ld(h0b, [32, 4], f32, "h0b")
            c0b_s = ld(c0b, [32, 4], f32, "c0b")
            p1wA_s = ld(p1wA, [128, 128], f32, "p1wA")
            p1wB_s = ld(p1wB, [128, 128], f32, "p1wB")
            p1wC_s = ld(p1wC, [98, 128], f32, "p1wC")
            p1b_s = ld(p1b, [128, 1], f32, "p1b")
            p2w_s = ld(p2w, [128, 128], f32, "p2w")
            p2b_s = ld(p2b, [128, 1], f32, "p2b")
            p3w_s = ld(p3w, [128, 16], f32, "p3w")
            p3b_s = ld(p3b, [16, 1], f32, "p3b")

            # ---- big persistent tensors ----
            xTa = bigp.tile([128, BC, T], f16, tag="bigA")  # x feats 0:128 (rows 0:66 used)
            xTb = bigp.tile([128, BC, T], f16, tag="bigB")  # x feats 66:194
            x1S = bigp.tile([33, T, 64], f16, tag="bigC")   # L0 out by step: [:, s, d*32+i]; row32=1
            h1S = bigp.tile([33, T, 64], f16, tag="bigD")   # L1 out by step

            nc.vector.memset(x1S[32:33, :, :], 1.0)
            nc.vector.memset(h1S[32:33, :, :], 1.0)

            # ---- x transpose loads ----
            for i in range(BC):
                nc.sync.dma_start_transpose(out=xTa[:, i, :], in_=x[i, :, 0:128])
                nc.sync.dma_start_transpose(out=xTb[:, i, :], in_=x[i, :, KA:DIM_TAG])

            # ---- init states (k: 0=l0f 1=l0b 2=l1f 3=l1b) ----
            h0t = [cp.tile([33, 64], f16, tag=f"h0t{l}", name=f"h0t{l}") for l in range(2)]
            nc.vector.memset(h0t[0][32:33, :], 1.0)
            nc.vector.memset(h0t[1][32:33, :], 1.0)
            cL = [psC.tile([32, 64], f32, tag=f"cL{l}", name=f"cL{l}") for l in range(2)]
            for k in range(4):
                xc = xcf_s if k % 2 == 0 else xcl_s
                ph = psT.tile([32, 32], f32, tag="pst")
                nc.tensor.matmul(ph, h0w_s[k], xc, start=True, stop=True)
                nc.scalar.activation(h0t[k // 2][0:32, (k % 2) * 32:(k % 2) * 32 + 32],
                                     ph, AF.Identity, bias=h0b_s[:, k:k + 1])
                pc = psT.tile([32, 32], f32, tag="pst")
                nc.tensor.matmul(pc, c0w_s[k], xc, start=True, stop=True)
                nc.scalar.activation(cL[k // 2][:, (k % 2) * 32:(k % 2) * 32 + 32],
                                     pc, AF.Identity, bias=c0b_s[:, k:k + 1])

            # ---- LSTM pass emitter ----
            def lstm_pass(outS, h0tile, c_ps, whf, whb, mms_f, mms_b, post_step=None):
                for s in range(T):
                    ps = psG.tile([128, 64], f32, tag="psg")
                    for d, (wh, mms) in enumerate(((whf, mms_f), (whb, mms_b))):
                        col = ps[:, d * 32:d * 32 + 32]
                        rhs_h = h0tile[:, d * 32:d * 32 + 32] if s == 0 \
                            else outS[0:33, s - 1, d * 32:d * 32 + 32]
                        nc.tensor.matmul(col, wh, rhs_h, start=True, stop=False)
                        lst = mms(s)
                        for j, (lw, rx) in enumerate(lst):
                            nc.tensor.matmul(col, lw, rx, start=False, stop=(j == len(lst) - 1))
                    sifo = wp.tile([96, 64], f32, tag="sifo")
                    nc.scalar.activation(sifo, ps[0:96, :], AF.Sigmoid)
                    tg = wp.tile([32, 64], f32, tag="tg")
                    nc.scalar.activation(tg, ps[96:128, :], AF.Tanh)
                    t1 = wp.tile([32, 64], f32, tag="t1")
                    nc.vector.tensor_mul(t1, sifo[0:32, :], tg)
                    t2 = wp.tile([32, 64], f32, tag="t2")
                    nc.vector.tensor_mul(t2, sifo[32:64, :], c_ps)
                    nc.vector.tensor_add(c_ps, t1, t2)
                    tch = wp.tile([32, 64], f32, tag="tch")
                    nc.scalar.activation(tch, c_ps, AF.Tanh)
                    nc.vector.tensor_mul(outS[0:32, s, :], sifo[64:96, :], tch)
                    if post_step is not None:
                        post_step(s)

            # ---- layer 0 ----
            l0f = lambda s: [(w0s["af"], xTa[0:KA, :, s]), (w0s["bf"], xTb[0:KB, :, s])]
            l0b = lambda s: [(w0s["ab"], xTa[0:KA, :, T - 1 - s]), (w0s["bb"], xTb[0:KB, :, T - 1 - s])]
            lstm_pass(x1S, h0t[0], cL[0], w0s["hf"], w0s["hb"], l0f, l0b)

            # ---- layer 1 (+ progressive hPf/hPb build) ----
            l1f = lambda s: [(w1s["ff"], x1S[0:32, s, 0:32]), (w1s["fb"], x1S[0:32, T - 1 - s, 32:64])]
            l1b = lambda s: [(w1s["bf"], x1S[0:32, T - 1 - s, 0:32]), (w1s["bb"], x1S[0:32, s, 32:64])]

            lstm_pass(h1S, h0t[1], cL[1], w1s["hf"], w1s["hb"], l1f, l1b)

            # attn-phase tensors reuse slots of xTa / x1S (released above)
            hS2 = bigp.tile([36, T, BC], f16, tag="bigA")  # rows0:4 attn h by s; row4=1; rows32:36 reversed
            hP = bigp.tile([128, 128, 65], f16, tag="bigC")  # per chunk: col0=1, 1:33 fwd-f, 33:65 bwd-f
            nc.vector.memset(hS2[4:5, :, :], 1.0)
            nc.vector.memset(hP[:, :, 0:1], 1.0)

            # ---- attn init: hn/cn -> h0a/c0a ----
            cn_sb = wp.tile([32, 64], f16, tag="cnsb")
            nc.vector.tensor_copy(cn_sb, cL[1])
            h0a = cp.tile([5, 32], f16, tag="h0a")
            nc.vector.memset(h0a[4:5, :], 1.0)
            cA = psC.tile([4, 32], f32, tag="cL0")
            for k in range(4):
                hn = h1S[0:32, T - 1, 32:64] if k < 2 else h1S[0:32, T - 1, 0:32]
                cn = cn_sb[:, 32:64] if k < 2 else cn_sb[:, 0:32]
                pa = psT.tile([1, 32], f32, tag="pst")
                nc.tensor.matmul(pa, adh_s[:, k:k + 1], hn, start=True, stop=True)
                nc.scalar.activation(h0a[k:k + 1, :], pa, AF.Identity, bias=adhb_s[:, k:k + 1])
                pcaa = psT.tile([1, 32], f32, tag="pst")
                nc.tensor.matmul(pcaa, adc_s[:, k:k + 1], cn, start=True, stop=True)
                nc.scalar.activation(cA[k:k + 1, :], pcaa, AF.Identity, bias=adcb_s[:, k:k + 1])

            # ---- attn scan (4 heads packed; gates [i,f,o,g] x head) ----
            for s in range(T):
                psA = psG.tile([16, 32], f32, tag="psg")
                rhs_h = h0a if s == 0 else hS2[0:5, s - 1, :]
                nc.tensor.matmul(psA, whha_s, rhs_h, start=True, stop=False)
                nc.tensor.matmul(psA, warf_s, h1S[0:32, s, 0:32], start=False, stop=False)
                nc.tensor.matmul(psA, warb_s, h1S[0:32, T - 1 - s, 32:64], start=False, stop=True)
                sA = wp.tile([12, 32], f32, tag="sA")
                nc.scalar.activation(sA, psA[0:12, :], AF.Sigmoid)
                tgA = wp.tile([4, 32], f32, tag="tgA")
                nc.scalar.activation(tgA, psA[12:16, :], AF.Tanh)
                t1a = wp.tile([4, 32], f32, tag="t1a")
                nc.vector.tensor_mul(t1a, sA[0:4, :], tgA)
                t2a = wp.tile([4, 32], f32, tag="t2a")
                nc.vector.tensor_mul(t2a, sA[4:8, :], cA)
                nc.vector.tensor_add(cA, t1a, t2a)
                tcA = wp.tile([4, 32], f32, tag="tcA")
                nc.scalar.activation(tcA, cA, AF.Tanh)
                nc.vector.tensor_mul(hS2[0:4, s, :], sA[8:12, :], tcA)
                nc.vector.tensor_copy(hS2[32:36, T - 1 - s, :], hS2[0:4, s, :])
                if s % 4 == 3:
                    k = s // 4
                    pf = psT.tile([128, 32], f16, tag="pst")
                    nc.tensor.transpose(pf, h1S[0:32, 4 * k:4 * k + 4, 0:32], idF_s[0:32, 0:32])
                    nc.scalar.activation(hP[:, k, 1:33], pf, AF.Copy)
                    pb = psT.tile([128, 32], f16, tag="pst")
                    nc.tensor.transpose(pb, h1S[0:32, 4 * k:4 * k + 4, 32:64], idF_s[0:32, 0:32])
                    nc.scalar.activation(hP[:, k, 33:65], pb, AF.Copy)

            # ---- softmax-pool: E chunks + accumulating matmuls ----
            poolF = psO.tile([33, 128], f32, tag="poolF")
            poolB = psO.tile([32, 128], f32, tag="poolB")
            NCH = T // 4
            for j in range(NCH):
                for (kk, srcl, srch, lhsl, lhsh, dst, first, last) in (
                    (j, 0, 4, 0, 33, poolF, j == 0, j == NCH - 1),
                    (NCH - 1 - j, 32, 36, 33, 65, poolB, j == 0, j == NCH - 1),
                ):
                    pe = psT.tile([128, 4], f16, tag="pst")
                    nc.tensor.transpose(pe, hS2[srcl:srch, 4 * kk:4 * kk + 4, :],
                                        idF_s[srcl:srch, srcl:srch])
                    eT = wp.tile([128, 4], f16, tag="eT")
                    nc.scalar.activation(eT, pe, AF.Exp)
                    E = wp.tile([128, 4, 32], f16, tag="Ech")
                    nc.vector.tensor_mul(
                        E, eT.rearrange("p (f o) -> p f o", o=1).broadcast_to([128, 4, 32]),
                        mask_s.rearrange("p (h i) -> p h i", h=4))
                    nc.tensor.matmul(dst, hP[:, kk, lhsl:lhsh], E.rearrange("p h i -> p (h i)"),
                                     start=first, stop=last, skip_group_check=True)

            # ---- pool assembly ----
            poolSb = wp.tile([65, 128], f16, tag="poolSb")
            nc.scalar.activation(poolSb[0:33, :], poolF, AF.Copy)
            nc.scalar.activation(poolSb[33:65, :], poolB, AF.Copy)
            ptp = psT.tile([128, 65], f16, tag="pst")
            nc.tensor.transpose(ptp, poolSb, idF_s[0:65, 0:65])
            poolT = wp.tile([128, 65], f32, tag="poolT")
            nc.vector.tensor_copy(poolT, ptp)
            rD = wp.tile([128, 1], f32, tag="rD")
            nc.vector.reciprocal(rD, poolT[:, 0:1])
            hcatA = wp.tile([128, 32], f32, tag="hcatA")
            hcatB = wp.tile([128, 32], f32, tag="hcatB")
            hcatC = wp.tile([98, 32], f32, tag="hcatC")
            for h in range(4):
                ph = wp.tile([32, 64], f32, tag="poolh")
                nc.vector.tensor_scalar_mul(ph, poolT[32 * h:32 * h + 32, 1:65], rD[32 * h:32 * h + 32, :])
                pt = psT.tile([64, 32], f32, tag="pst")
                nc.tensor.transpose(pt, ph, idS_s)
                dstt = (hcatA, hcatB)[h // 2]
                nc.scalar.activation(dstt[(h % 2) * 64:(h % 2) * 64 + 64, :], pt, AF.Copy)
            nc.vector.tensor_copy(hcatC[0:DIM_COM, :], xcf_s)
            nc.vector.tensor_copy(hcatC[DIM_COM:2 * DIM_COM, :], xcl_s)

            # ---- MLP head ----
            z1 = psO.tile([128, 32], f32, tag="poolF")
            nc.tensor.matmul(z1, p1wA_s, hcatA, start=True, stop=False)
            nc.tensor.matmul(z1, p1wB_s, hcatB, start=False, stop=False)
            nc.tensor.matmul(z1, p1wC_s, hcatC, start=False, stop=True)
            s1 = wp.tile([128, 32], f32, tag="s1")
            nc.scalar.activation(s1, z1, AF.Relu, bias=p1b_s[:, 0:1])
            z2 = psO.tile([128, 32], f32, tag="poolB")
            nc.tensor.matmul(z2, p2w_s, s1, start=True, stop=True)
            s2 = wp.tile([128, 32], f32, tag="s2")
            nc.scalar.activation(s2, z2, AF.Relu, bias=p2b_s[:, 0:1])
            z3 = psO.tile([16, 32], f32, tag="poolF")
            nc.tensor.matmul(z3, p3w_s, s2, start=True, stop=True)
            ysb = wp.tile([16, 32], f32, tag="ysb")
            nc.scalar.activation(ysb, z3, AF.Sigmoid, bias=p3b_s[:, 0:1])
            nc.sync.dma_start(out=y[:, :], in_=ysb)

            if debug:
                nc.sync.dma_start(out=dbg["x1S"], in_=x1S)
                nc.sync.dma_start(out=dbg["h1S"], in_=h1S)
                nc.sync.dma_start(out=dbg["hS"], in_=hS2[0:5, :, :])
                nc.sync.dma_start(out=dbg["poolT"], in_=poolT)
    return nc


_NC_CACHE = {}


def _get_nc(debug=False):
    if debug not in _NC_CACHE:
        _NC_CACHE[debug] = _build_nc(debug)
    return _NC_CACHE[debug]


def _prep_weights(pre_w, pre_b, h0_w, h0_b, c0_w, c0_b, rnn0_wih, rnn1_wih,
                  rnn_whh, rnn_bih, rnn_bhh, adh_w, adh_b, adc_w, adc_b,
                  ar_wih, ar_whh, ar_bih, ar_bhh, p1_w, p1_b, p2_w, p2_b, p3_w, p3_b):
    f, f16 = np.float32, np.float16
    P = _perm128()
    go = [0, 1, 3, 2]  # new gate order [i,f,o,g] -> orig index
    wm = {}
    for d, dn in ((0, "f"), (1, "b")):
        W0 = (rnn0_wih[d].astype(f) @ pre_w.astype(f))[P]      # [128,194]
        b0 = (rnn0_wih[d].astype(f) @ pre_b.astype(f) + rnn_bih[0, d] + rnn_bhh[0, d])[P]
        wm["w0a" + dn] = np.ascontiguousarray(W0[:, 0:KA].T).astype(f16)
        wm["w0b" + dn] = np.ascontiguousarray(W0[:, KA:DIM_TAG].T).astype(f16)
        wh = np.concatenate([rnn_whh[0, d][P].T.astype(f), b0[None, :]], axis=0)
        wm["whh0" + dn] = wh.astype(f16)
        W1 = rnn1_wih[d][P].astype(f)                          # [128,64]
        b1 = (rnn_bih[1, d] + rnn_bhh[1, d])[P]
        wm["w1f" + dn] = np.ascontiguousarray(W1[:, 0:32].T).astype(f16)
        wm["w1b" + dn] = np.ascontiguousarray(W1[:, 32:64].T).astype(f16)
        wh1 = np.concatenate([rnn_whh[1, d][P].T.astype(f), b1[None, :]], axis=0)
        wm["whh1" + dn] = wh1.astype(f16)
    warf = np.zeros((32, 16), f)
    warb = np.zeros((32, 16), f)
    whha = np.zeros((5, 16), f)
    for g in range(4):
        for k in range(4):
            m = g * 4 + k
            warf[:, m] = ar_wih[k, go[g], 0:32]
            warb[:, m] = ar_wih[k, go[g], 32:64]
            whha[k, m] = ar_whh[k, go[g], 0]
            whha[4, m] = ar_bih[k, go[g]] + ar_bhh[k, go[g]]
    wm["warf"], wm["warb"], wm["whha"] = warf.astype(f16), warb.astype(f16), whha.astype(f16)
    wm["adh"] = np.ascontiguousarray(adh_w.T).astype(f16)
    wm["adc"] = np.ascontiguousarray(adc_w.T).astype(f16)
    wm["adhb"] = adh_b.reshape(1, 4).astype(f)
    wm["adcb"] = adc_b.reshape(1, 4).astype(f)
    for k in range(4):
        wm[f"h0w{k}"] = np.ascontiguousarray(h0_w[k].T).astype(f)
        wm[f"c0w{k}"] = np.ascontiguousarray(c0_w[k].T).astype(f)
    wm["h0b"] = np.ascontiguousarray(h0_b.T).astype(f)
    wm["c0b"] = np.ascontiguousarray(c0_b.T).astype(f)
    p1T = np.ascontiguousarray(p1_w.T).astype(f)  # [354,128]
    wm["p1wA"] = p1T[0:128]
    wm["p1wB"] = p1T[128:256]
    wm["p1wC"] = np.ascontiguousarray(p1T[256:354])
    wm["p1b"] = p1_b.reshape(-1, 1).astype(f)
    wm["p2w"] = np.ascontiguousarray(p2_w.T).astype(f)
    wm["p2b"] = p2_b.reshape(-1, 1).astype(f)
    wm["p3w"] = np.ascontiguousarray(p3_w.T).astype(f)
    wm["p3b"] = p3_b.reshape(-1, 1).astype(f)
    wm["idF"] = np.eye(128, dtype=f16)
    wm["idS"] = np.eye(32, dtype=f)
    msk = np.zeros((128, 128), f16)
    for p in range(128):
        for hh in range(4):
            msk[p, hh * 32 + p % 32] = 1.0
    wm["mask"] = msk
    return wm


def _run_device(x_tag, x_com_first, x_com_last, wm, debug=False):
    from concourse import bass_utils
    nc = _get_nc(debug)
    x16 = x_tag.astype(np.float16)
    xcfT = x_com_first.astype(np.float32).T  # [49, 256]
    xclT = x_com_last.astype(np.float32).T
    in_maps = []
    for c in range(N_CORES):
        m = dict(wm)
        m["x"] = x16[c * BC:(c + 1) * BC]
        m["xcf"] = np.ascontiguousarray(xcfT[:, c * BC:(c + 1) * BC])
        m["xcl"] = np.ascontiguousarray(xclT[:, c * BC:(c + 1) * BC])
        in_maps.append(m)
    res = bass_utils.run_bass_kernel_spmd(nc, in_maps, core_ids=list(range(N_CORES)))
    out = np.empty((B, 16), np.float32)
    for c in range(N_CORES):
        out[c * BC:(c + 1) * BC] = res.results[c]["y"].T
    if debug:
        return out, res
    return out


def kernel(x_tag, x_com_first, x_com_last, pre_w, pre_b, h0_w, h0_b, c0_w, c0_b,
           rnn0_wih, rnn1_wih, rnn_whh, rnn_bih, rnn_bhh,
           adh_w, adh_b, adc_w, adc_b, ar_wih, ar_whh, ar_bih, ar_bhh,
           p1_w, p1_b, p2_w, p2_b, p3_w, p3_b):
    args = dict(pre_w=pre_w, pre_b=pre_b, h0_w=h0_w, h0_b=h0_b, c0_w=c0_w, c0_b=c0_b,
                rnn0_wih=rnn0_wih, rnn1_wih=rnn1_wih, rnn_whh=rnn_whh,
                rnn_bih=rnn_bih, rnn_bhh=rnn_bhh, adh_w=adh_w, adh_b=adh_b,
                adc_w=adc_w, adc_b=adc_b, ar_wih=ar_wih, ar_whh=ar_whh,
                ar_bih=ar_bih, ar_bhh=ar_bhh, p1_w=p1_w, p1_b=p1_b,
                p2_w=p2_w, p2_b=p2_b, p3_w=p3_w, p3_b=p3_b)
    wm = _prep_weights(**{k: np.asarray(v) for k, v in args.items()})
    try:
        return _run_device(np.asarray(x_tag, np.float32),
                           np.asarray(x_com_first, np.float32),
                           np.asarray(x_com_last, np.float32), wm)
    except Exception:
        import traceback
        traceback.print_exc()
        return _kernel_numpy(np.asarray(x_tag, np.float32),
                             np.asarray(x_com_first, np.float32),
                             np.asarray(x_com_last, np.float32),
                             **{k: np.asarray(v, np.float32) for k, v in args.items()})


def _sigmoid(x):
    return 1.0 / (1.0 + np.exp(-x))


def _lstm_np(pre, h, c, whh, bhh, reverse=False):
    Bq, Tq, G = pre.shape
    Hh = G // 4
    whh_T = whh.T.astype(np.float32)
    hs = np.empty((Bq, Tq, Hh), np.float32)
    ts = range(Tq - 1, -1, -1) if reverse else range(Tq)
    for t in ts:
        g = pre[:, t] + h @ whh_T + bhh
        i, f, gg, o = np.split(g, 4, axis=-1)
        c = _sigmoid(f) * c + _sigmoid(i) * np.tanh(gg)
        h = _sigmoid(o) * np.tanh(c)
        hs[:, t] = h
    return hs, h, c


def _kernel_numpy(x_tag, x_com_first, x_com_last, pre_w, pre_b, h0_w, h0_b, c0_w, c0_b,
                  rnn0_wih, rnn1_wih, rnn_whh, rnn_bih, rnn_bhh,
                  adh_w, adh_b, adc_w, adc_b, ar_wih, ar_whh, ar_bih, ar_bhh,
                  p1_w, p1_b, p2_w, p2_b, p3_w, p3_b):
    f = np.float32
    xc = np.stack([x_com_first, x_com_last, x_com_first, x_com_last]).astype(f)
    h0 = np.einsum('kbd,khd->kbh', xc, h0_w).astype(f) + h0_b[:, None, :]
    c0 = np.einsum('kbd,khd->kbh', xc, c0_w).astype(f) + c0_b[:, None, :]
    h_tag = (x_tag.reshape(-1, DIM_TAG) @ pre_w.T).reshape(B, T, H) + pre_b

    def inproj(xseq, wih, bih):
        r = xseq.reshape(B * T, -1) @ wih.T.astype(f) + bih
        return r.reshape(B, T, -1).astype(f)

    hf0, _, _ = _lstm_np(inproj(h_tag, rnn0_wih[0], rnn_bih[0, 0]), h0[0], c0[0], rnn_whh[0, 0], rnn_bhh[0, 0], False)
    hb0, _, _ = _lstm_np(inproj(h_tag, rnn0_wih[1], rnn_bih[0, 1]), h0[1], c0[1], rnn_whh[0, 1], rnn_bhh[0, 1], True)
    x1 = np.concatenate([hf0, hb0], axis=-1)
    hf1, hnf, cnf = _lstm_np(inproj(x1, rnn1_wih[0], rnn_bih[1, 0]), h0[2], c0[2], rnn_whh[1, 0], rnn_bhh[1, 0], False)
    hb1, hnb, cnb = _lstm_np(inproj(x1, rnn1_wih[1], rnn_bih[1, 1]), h0[3], c0[3], rnn_whh[1, 1], rnn_bhh[1, 1], True)
    h_out = np.concatenate([hf1, hb1], axis=-1)
    hn_sel = np.stack([hnb, hnb, hnf, hnf])
    cn_sel = np.stack([cnb, cnb, cnf, cnf])
    h0a = (np.einsum('kbd,kd->kb', hn_sel, adh_w).astype(f) + adh_b[:, None])[..., None]
    c0a = (np.einsum('kbd,kd->kb', cn_sel, adc_w).astype(f) + adc_b[:, None])[..., None]
    attn = np.empty((4, B, T, 1), f)
    for k in range(4):
        attn[k] = _lstm_np(inproj(h_out, ar_wih[k], ar_bih[k]), h0a[k], c0a[k], ar_whh[k], ar_bhh[k], False)[0]
    attn = np.exp(attn - attn.max(axis=2, keepdims=True))
    attn = attn / attn.sum(axis=2, keepdims=True)
    pooled = np.sum(attn * h_out[None], axis=2)
    hh = np.concatenate([pooled[0], pooled[1], pooled[2], pooled[3], x_com_first, x_com_last], axis=1).astype(f)
    hh = np.maximum(hh @ p1_w.T.astype(f) + p1_b, 0.0)
    hh = np.maximum(hh @ p2_w.T.astype(f) + p2_b, 0.0)
    return _sigmoid(hh @ p3_w.T.astype(f) + p3_b).astype(f)


# revision 26
# speedup vs baseline: 3.0694x; 3.0694x over previous
import sys
import numpy as np

sys.path.insert(0, "/opt/trn_rl_repo")

B, T = 256, 512
DIM_TAG, DIM_COM, H = 194, 49, 32
N_CORES = 8
BC = B // N_CORES  # 32 batch rows per core
KA, KB = 66, 128   # feature split of 194 = 66 + 128

_PERM128 = None


def _perm128():
    # reorder gate rows [i,f,g,o] (PyTorch) -> [i,f,o,g] so sigmoid rows are 0:96
    global _PERM128
    if _PERM128 is None:
        _PERM128 = np.concatenate([np.arange(0, 64), np.arange(96, 128), np.arange(64, 96)])
    return _PERM128


def _build_nc(debug=False):
    import concourse.bass as bass
    import concourse.mybir as mybir
    from concourse.tile import TileContext

    f32 = mybir.dt.float32
    f16 = mybir.dt.float16
    AF = mybir.ActivationFunctionType

    nc = bass.Bass()
    DT = nc.dram_tensor

    x = DT("x", [BC, T, DIM_TAG], f16, kind="ExternalInput")
    xcf = DT("xcf", [DIM_COM, BC], f32, kind="ExternalInput")
    xcl = DT("xcl", [DIM_COM, BC], f32, kind="ExternalInput")
    idF = DT("idF", [128, 128], f16, kind="ExternalInput")
    idS = DT("idS", [64, 64], f32, kind="ExternalInput")
    mask = DT("mask", [128, 128], f16, kind="ExternalInput")
    # L0 input weights (pre_w folded), K split 0:66 / 66:194
    w0 = {}
    for d in ("f", "b"):
        w0["a" + d] = DT("w0a" + d, [KA, 128], f16, kind="ExternalInput")
        w0["b" + d] = DT("w0b" + d, [KB, 128], f16, kind="ExternalInput")
        w0["h" + d] = DT("whh0" + d, [33, 128], f16, kind="ExternalInput")
    w1 = {}
    for d in ("f", "b"):
        w1["f" + d] = DT("w1f" + d, [32, 128], f16, kind="ExternalInput")
        w1["b" + d] = DT("w1b" + d, [32, 128], f16, kind="ExternalInput")
        w1["h" + d] = DT("whh1" + d, [33, 128], f16, kind="ExternalInput")
    warf = DT("warf", [32, 100], f16, kind="ExternalInput")
    warb = DT("warb", [32, 100], f16, kind="ExternalInput")
    whha = DT("whha", [5, 100], f16, kind="ExternalInput")
    adh = DT("adh", [32, 4], f16, kind="ExternalInput")
    adc = DT("adc", [32, 4], f16, kind="ExternalInput")
    adhb = DT("adhb", [1, 4], f32, kind="ExternalInput")
    adcb = DT("adcb", [1, 4], f32, kind="ExternalInput")
    h0w = [DT(f"h0w{k}", [DIM_COM, 32], f32, kind="ExternalInput") for k in range(4)]
    c0w = [DT(f"c0w{k}", [DIM_COM, 32], f32, kind="ExternalInput") for k in range(4)]
    h0b = DT("h0b", [32, 4], f32, kind="ExternalInput")
    c0b = DT("c0b", [32, 4], f32, kind="ExternalInput")
    p1wA = DT("p1wA", [128, 128], f32, kind="ExternalInput")
    p1wB = DT("p1wB", [128, 128], f32, kind="ExternalInput")
    p1wC = DT("p1wC", [98, 128], f32, kind="ExternalInput")
    p1b = DT("p1b", [128, 1], f32, kind="ExternalInput")
    p2w = DT("p2w", [128, 128], f32, kind="ExternalInput")
    p2b = DT("p2b", [128, 1], f32, kind="ExternalInput")
    p3w = DT("p3w", [128, 16], f32, kind="ExternalInput")
    p3b = DT("p3b", [16, 1], f32, kind="ExternalInput")
    y = DT("y", [16, BC], f32, kind="ExternalOutput")
    dbg = {}
    if debug:
        dbg["x1S"] = DT("dbg_x1S", [33, T, 64], f16, kind="ExternalOutput")
        dbg["h1S"] = DT("dbg_h1S", [33, T, 64], f16, kind="ExternalOutput")
        dbg["hS"] = DT("dbg_hS", [5, T, BC], f16, kind="ExternalOutput")
        dbg["poolT"] = DT("dbg_poolT", [128, 65], f32, kind="ExternalOutput")

    with TileContext(nc) as tc:
        with (
            tc.tile_pool(name="const", bufs=1) as cp,
            tc.tile_pool(name="big", bufs=1) as bigp,
            tc.tile_pool(name="work", bufs=2) as wp,
            tc.tile_pool(name="psG", bufs=2, space="PSUM") as psG,
            tc.tile_pool(name="psC", bufs=1, space="PSUM") as psC,
            tc.tile_pool(name="psT", bufs=2, space="PSUM") as psT,
            tc.tile_pool(name="psO", bufs=1, space="PSUM") as psO,
        ):
            def ld(drt, shape, dt, tag):
                t = cp.tile(shape, dt, tag=tag, name="ld_" + tag)
                nc.sync.dma_start(out=t, in_=drt[:, :] if len(shape) == 2 else drt)
                return t

            # ---- constants to SBUF ----
            idF_s = ld(idF, [128, 128], f16, "idF")
            idS_s = ld(idS, [64, 64], f32, "idS")
            mask_s = ld(mask, [128, 128], f16, "mask")
            xcf_s = ld(xcf, [DIM_COM, BC], f32, "xcf")
            xcl_s = ld(xcl, [DIM_COM, BC], f32, "xcl")
            w0s = {k: ld(v, list(v.shape), f16, "w0" + k) for k, v in w0.items()}
            w1s = {k: ld(v, list(v.shape), f16, "w1" + k) for k, v in w1.items()}
            warf_s = ld(warf, [32, 100], f16, "warf")
            warb_s = ld(warb, [32, 100], f16, "warb")
            whha_s = ld(whha, [5, 100], f16, "whha")
            adh_s = ld(adh, [32, 4], f16, "adh")
            adc_s = ld(adc, [32, 4], f16, "adc")
            adhb_s = ld(adhb, [1, 4], f32, "adhb")
            adcb_s = ld(adcb, [1, 4], f32, "adcb")
            h0w_s = [ld(h0w[k], [DIM_COM, 32], f32, f"h0w{k}") for k in range(4)]
            c0w_s = [ld(c0w[k], [DIM_COM, 32], f32, f"c0w{k}") for k in range(4)]
            h0b_s = ld(h0b, [32, 4], f32, "h0b")
            c0b_s = ld(c0b, [32, 4], f32, "c0b")
            p1wA_s = ld(p1wA, [128, 128], f32, "p1wA")
            p1wB_s = ld(p1wB, [128, 128], f32, "p1wB")
            p1wC_s = ld(p1wC, [98, 128], f32, "p1wC")
            p1b_s = ld(p1b, [128, 1], f32, "p1b")
            p2w_s = ld(p2w, [128, 128], f32, "p2w")
            p2b_s = ld(p2b, [128, 1], f32, "p2b")
            p3w_s = ld(p3w, [128, 16], f32, "p3w")
            p3b_s = ld(p3b, [16, 1], f32, "p3b")

            # ---- big persistent tensors ----
            xTa = bigp.tile([128, BC, T], f16, tag="bigA")  # x feats 0:128 (rows 0:66 used)
            xTb = bigp.tile([128, BC, T], f16, tag="bigB")  # x feats 66:194
            x1S = bigp.tile([33, T, 64], f16, tag="bigC")   # L0 out by step: [:, s, d*32+i]; row32=1
            h1S = bigp.tile([33, T, 64], f16, tag="bigD")   # L1 out by step

            nc.vector.memset(x1S[32:33, :, :], 1.0)
            nc.vector.memset(h1S[32:33, :, :], 1.0)

            # ---- x transpose loads ----
            for i in range(BC):
                nc.sync.dma_start_transpose(out=xTa[:, i, :], in_=x[i, :, 0:128])
                nc.sync.dma_start_transpose(out=xTb[:, i, :], in_=x[i, :, KA:DIM_TAG])

            # ---- init states (k: 0=l0f 1=l0b 2=l1f 3=l1b) ----
            h0t = [cp.tile([33, 64], f16, tag=f"h0t{l}", name=f"h0t{l}") for l in range(2)]
            nc.vector.memset(h0t[0][32:33, :], 1.0)
            nc.vector.memset(h0t[1][32:33, :], 1.0)
            cL = [cp.tile([32, 64], f32, tag=f"cL{l}", name=f"cL{l}") for l in range(2)]
            for k in range(4):
                xc = xcf_s if k % 2 == 0 else xcl_s
                ph = psT.tile([32, 32], f32, tag="pst")
                nc.tensor.matmul(ph, h0w_s[k], xc, start=True, stop=True)
                nc.scalar.activation(h0t[k // 2][0:32, (k % 2) * 32:(k % 2) * 32 + 32],
                                     ph, AF.Identity, bias=h0b_s[:, k:k + 1])
                pc = psT.tile([32, 32], f32, tag="pst")
                nc.tensor.matmul(pc, c0w_s[k], xc, start=True, stop=True)
                nc.scalar.activation(cL[k // 2][:, (k % 2) * 32:(k % 2) * 32 + 32],
                                     pc, AF.Identity, bias=c0b_s[:, k:k + 1])

            # ---- LSTM pass emitter ----
            def lstm_pass(outS, h0tile, c_sb, whf, whb, mms_f, mms_b, post_step=None):
                for s in range(T):
                    ps = psG.tile([128, 64], f32, tag="psg")
                    for d, (wh, mms) in enumerate(((whf, mms_f), (whb, mms_b))):
                        col = ps[:, d * 32:d * 32 + 32]
                        rhs_h = h0tile[:, d * 32:d * 32 + 32] if s == 0 \
                            else outS[0:33, s - 1, d * 32:d * 32 + 32]
                        nc.tensor.matmul(col, wh, rhs_h, start=True, stop=False)
                        lst = mms(s)
                        for j, (lw, rx) in enumerate(lst):
                            nc.tensor.matmul(col, lw, rx, start=False, stop=(j == len(lst) - 1))
                    nc.scalar.activation(ps[0:96, :], ps[0:96, :], AF.Sigmoid)
                    tg = wp.tile([32, 64], f32, tag="tg")
                    nc.scalar.activation(tg, ps[96:128, :], AF.Tanh)
                    t1 = wp.tile([32, 64], f32, tag="t1")
                    nc.vector.tensor_mul(t1, ps[0:32, :], tg)
                    t2 = wp.tile([32, 64], f32, tag="t2")
                    nc.vector.tensor_mul(t2, ps[32:64, :], c_sb)
                    nc.vector.tensor_add(c_sb, t1, t2)
                    tch = wp.tile([32, 64], f32, tag="tch")
                    nc.scalar.activation(tch, c_sb, AF.Tanh)
                    nc.vector.tensor_mul(outS[0:32, s, :], ps[64:96, :], tch)
                    if post_step is not None:
                        post_step(s)

            # ---- layer 0 ----
            l0f = lambda s: [(w0s["af"], xTa[0:KA, :, s]), (w0s["bf"], xTb[0:KB, :, s])]
            l0b = lambda s: [(w0s["ab"], xTa[0:KA, :, T - 1 - s]), (w0s["bb"], xTb[0:KB, :, T - 1 - s])]
            lstm_pass(x1S, h0t[0], cL[0], w0s["hf"], w0s["hb"], l0f, l0b)

            # ---- layer 1 (+ progressive hPf/hPb build) ----
            l1f = lambda s: [(w1s["ff"], x1S[0:32, s, 0:32]), (w1s["fb"], x1S[0:32, T - 1 - s, 32:64])]
            l1b = lambda s: [(w1s["bf"], x1S[0:32, T - 1 - s, 0:32]), (w1s["bb"], x1S[0:32, s, 32:64])]

            lstm_pass(h1S, h0t[1], cL[1], w1s["hf"], w1s["hb"], l1f, l1b)

            # attn-phase tensors reuse slots of xTa / x1S (released above)
            hS2 = bigp.tile([36, T, BC], f16, tag="bigA")  # rows0:4 attn h by s; row4=1; rows32:36 reversed
            hP = bigp.tile([128, 128, 65], f16, tag="bigC")  # per chunk: col0=1, 1:33 fwd-f, 33:65 bwd-f
            nc.vector.memset(hS2[4:5, :, :], 1.0)
            nc.vector.memset(hP[:, :, 0:1], 1.0)

            # ---- attn init: hn/cn -> h0a/c0a ----
            cn_sb = cp.tile([32, 64], f16, tag="cnsb")
            nc.vector.tensor_copy(cn_sb, cL[1])
            h0a = cp.tile([5, 32], f16, tag="h0a")
            nc.vector.memset(h0a[4:5, :], 1.0)
            cA = cp.tile([4, 32], f32, tag="cA", name="cA")
            for k in range(4):
                hn = h1S[0:32, T - 1, 32:64] if k < 2 else h1S[0:32, T - 1, 0:32]
                cn = cn_sb[:, 32:64] if k < 2 else cn_sb[:, 0:32]
                pa = psT.tile([1, 32], f32, tag="pst")
                nc.tensor.matmul(pa, adh_s[:, k:k + 1], hn, start=True, stop=True)
                nc.scalar.activation(h0a[k:k + 1, :], pa, AF.Identity, bias=adhb_s[:, k:k + 1])
                pcaa = psT.tile([1, 32], f32, tag="pst")
                nc.tensor.matmul(pcaa, adc_s[:, k:k + 1], cn, start=True, stop=True)
                nc.scalar.activation(cA[k:k + 1, :], pcaa, AF.Identity, bias=adcb_s[:, k:k + 1])

            # ---- attn scan (4 heads packed; gates [i,f,o,g] x head) ----
            for s in range(T):
                psA = psG.tile([100, 32], f32, tag="psg")
                rhs_h = h0a if s == 0 else hS2[0:5, s - 1, :]
                nc.tensor.matmul(psA, whha_s, rhs_h, start=True, stop=False)
                nc.tensor.matmul(psA, warf_s, h1S[0:32, s, 0:32], start=False, stop=False)
                nc.tensor.matmul(psA, warb_s, h1S[0:32, T - 1 - s, 32:64], start=False, stop=True)
                nc.scalar.activation(psA[0:68, :], psA[0:68, :], AF.Sigmoid)
                tgA = wp.tile([4, 32], f32, tag="tgA")
                nc.scalar.activation(tgA, psA[96:100, :], AF.Tanh)
                t1a = wp.tile([4, 32], f32, tag="t1a")
                nc.vector.tensor_mul(t1a, psA[0:4, :], tgA)
                t2a = wp.tile([4, 32], f32, tag="t2a")
                nc.vector.tensor_mul(t2a, psA[32:36, :], cA)
                nc.vector.tensor_add(cA, t1a, t2a)
                tcA = wp.tile([4, 32], f32, tag="tcA")
                nc.scalar.activation(tcA, cA, AF.Tanh)
                nc.vector.tensor_mul(hS2[0:4, s, :], psA[64:68, :], tcA)
                nc.scalar.activation(hS2[32:36, T - 1 - s, :], hS2[0:4, s, :], AF.Copy)
                if s % 4 == 3:
                    k = s // 4
                    pf = psT.tile([128, 32], f16, tag="pst")
                    nc.tensor.transpose(pf, h1S[0:32, 4 * k:4 * k + 4, 0:32], idF_s[0:32, 0:32])
                    nc.scalar.activation(hP[:, k, 1:33], pf, AF.Copy)
                    pb = psT.tile([128, 32], f16, tag="pst")
                    nc.tensor.transpose(pb, h1S[0:32, 4 * k:4 * k + 4, 32:64], idF_s[0:32, 0:32])
                    nc.scalar.activation(hP[:, k, 33:65], pb, AF.Copy)

            # ---- softmax-pool: E chunks + accumulating matmuls ----
            poolF = psO.tile([33, 128], f32, tag="poolF")
            poolB = psO.tile([32, 128], f32, tag="poolB")
            NCH = T // 4
            for j in range(NCH):
                for (kk, srcl, srch, lhsl, lhsh, dst, first, last) in (
                    (j, 0, 4, 0, 33, poolF, j == 0, j == NCH - 1),
                    (NCH - 1 - j, 32, 36, 33, 65, poolB, j == 0, j == NCH - 1),
                ):
                    pe = psT.tile([128, 4], f16, tag="pst")
                    nc.tensor.transpose(pe, hS2[srcl:srch, 4 * kk:4 * kk + 4, :],
                                        idF_s[srcl:srch, srcl:srch])
                    eT = wp.tile([128, 4], f16, tag="eT")
                    nc.scalar.activation(eT, pe, AF.Exp)
                    E = wp.tile([128, 4, 32], f16, tag="Ech")
                    nc.vector.tensor_mul(
                        E, eT.rearrange("p (f o) -> p f o", o=1).broadcast_to([128, 4, 32]),
                        mask_s.rearrange("p (h i) -> p h i", h=4))
                    nc.tensor.matmul(dst, hP[:, kk, lhsl:lhsh], E.rearrange("p h i -> p (h i)"),
                                     start=first, stop=last, skip_group_check=True)

            # ---- pool assembly ----
            poolSb = cp.tile([65, 128], f16, tag="poolSb")
            nc.scalar.activation(poolSb[0:33, :], poolF, AF.Copy)
            nc.scalar.activation(poolSb[33:65, :], poolB, AF.Copy)
            ptp = psT.tile([128, 65], f16, tag="pst")
            nc.tensor.transpose(ptp, poolSb, idF_s[0:65, 0:65])
            poolT = cp.tile([128, 65], f32, tag="poolT")
            nc.vector.tensor_copy(poolT, ptp)
            rD = cp.tile([128, 1], f32, tag="rD")
            nc.vector.reciprocal(rD, poolT[:, 0:1])
            hcatA = cp.tile([128, 32], f32, tag="hcatA")
            hcatB = cp.tile([128, 32], f32, tag="hcatB")
            hcatC = cp.tile([98, 32], f32, tag="hcatC")
            pscl = cp.tile([128, 64], f32, tag="pscl")
            nc.vector.tensor_scalar_mul(pscl, poolT[:, 1:65], rD)
            pscl2 = cp.tile([64, 64], f32, tag="pscl2")
            nc.scalar.activation(pscl2, pscl[64:128, :], AF.Copy)
            for h in range(4):
                src_ap = (pscl, pscl2)[h // 2][(h % 2) * 32:(h % 2) * 32 + 32, :]
                pt = psT.tile([64, 32], f32, tag="pst")
                idsl = idS_s[0:32, 0:32] if h % 2 == 0 else idS_s[32:64, 32:64]
                nc.tensor.transpose(pt, src_ap, idsl)
                dstt = (hcatA, hcatB)[h // 2]
                nc.scalar.activation(dstt[(h % 2) * 64:(h % 2) * 64 + 64, :], pt, AF.Copy)
            nc.scalar.activation(hcatC[0:DIM_COM, :], xcf_s, AF.Copy)
            nc.scalar.activation(hcatC[DIM_COM:2 * DIM_COM, :], xcl_s, AF.Copy)

            # ---- MLP head ----
            z1 = psO.tile([128, 32], f32, tag="poolF")
            nc.tensor.matmul(z1, p1wA_s, hcatA, start=True, stop=False)
            nc.tensor.matmul(z1, p1wB_s, hcatB, start=False, stop=False)
            nc.tensor.matmul(z1, p1wC_s, hcatC, start=False, stop=True)
            s1 = cp.tile([128, 32], f32, tag="s1")
            nc.scalar.activation(s1, z1, AF.Relu, bias=p1b_s[:, 0:1])
            z2 = psO.tile([128, 32], f32, tag="poolB")
            nc.tensor.matmul(z2, p2w_s, s1, start=True, stop=True)
            s2 = cp.tile([128, 32], f32, tag="s2")
            nc.scalar.activation(s2, z2, AF.Relu, bias=p2b_s[:, 0:1])
            z3 = psO.tile([16, 32], f32, tag="poolF")
            nc.tensor.matmul(z3, p3w_s, s2, start=True, stop=True)
            ysb = cp.tile([16, 32], f32, tag="ysb")
            nc.scalar.activation(ysb, z3, AF.Sigmoid, bias=p3b_s[:, 0:1])
            nc.sync.dma_start(out=y[:, :], in_=ysb)

            if debug:
                nc.sync.dma_start(out=dbg["x1S"], in_=x1S)
                nc.sync.dma_start(out=dbg["h1S"], in_=h1S)
                nc.sync.dma_start(out=dbg["hS"], in_=hS2[0:5, :, :])
                nc.sync.dma_start(out=dbg["poolT"], in_=poolT)
    return nc


_NC_CACHE = {}


def _get_nc(debug=False):
    if debug not in _NC_CACHE:
        _NC_CACHE[debug] = _build_nc(debug)
    return _NC_CACHE[debug]


def _prep_weights(pre_w, pre_b, h0_w, h0_b, c0_w, c0_b, rnn0_wih, rnn1_wih,
                  rnn_whh, rnn_bih, rnn_bhh, adh_w, adh_b, adc_w, adc_b,
                  ar_wih, ar_whh, ar_bih, ar_bhh, p1_w, p1_b, p2_w, p2_b, p3_w, p3_b):
    f, f16 = np.float32, np.float16
    P = _perm128()
    go = [0, 1, 3, 2]  # new gate order [i,f,o,g] -> orig index
    wm = {}
    for d, dn in ((0, "f"), (1, "b")):
        W0 = (rnn0_wih[d].astype(f) @ pre_w.astype(f))[P]      # [128,194]
        b0 = (rnn0_wih[d].astype(f) @ pre_b.astype(f) + rnn_bih[0, d] + rnn_bhh[0, d])[P]
        wm["w0a" + dn] = np.ascontiguousarray(W0[:, 0:KA].T).astype(f16)
        wm["w0b" + dn] = np.ascontiguousarray(W0[:, KA:DIM_TAG].T).astype(f16)
        wh = np.concatenate([rnn_whh[0, d][P].T.astype(f), b0[None, :]], axis=0)
        wm["whh0" + dn] = wh.astype(f16)
        W1 = rnn1_wih[d][P].astype(f)                          # [128,64]
        b1 = (rnn_bih[1, d] + rnn_bhh[1, d])[P]
        wm["w1f" + dn] = np.ascontiguousarray(W1[:, 0:32].T).astype(f16)
        wm["w1b" + dn] = np.ascontiguousarray(W1[:, 32:64].T).astype(f16)
        wh1 = np.concatenate([rnn_whh[1, d][P].T.astype(f), b1[None, :]], axis=0)
        wm["whh1" + dn] = wh1.astype(f16)
    warf = np.zeros((32, 100), f)
    warb = np.zeros((32, 100), f)
    whha = np.zeros((5, 100), f)
    for g in range(4):
        for k in range(4):
            m = g * 32 + k
            warf[:, m] = ar_wih[k, go[g], 0:32]
            warb[:, m] = ar_wih[k, go[g], 32:64]
            whha[k, m] = ar_whh[k, go[g], 0]
            whha[4, m] = ar_bih[k, go[g]] + ar_bhh[k, go[g]]
    wm["warf"], wm["warb"], wm["whha"] = warf.astype(f16), warb.astype(f16), whha.astype(f16)
    wm["adh"] = np.ascontiguousarray(adh_w.T).astype(f16)
    wm["adc"] = np.ascontiguousarray(adc_w.T).astype(f16)
    wm["adhb"] = adh_b.reshape(1, 4).astype(f)
    wm["adcb"] = adc_b.reshape(1, 4).astype(f)
    for k in range(4):
        wm[f"h0w{k}"] = np.ascontiguousarray(h0_w[k].T).astype(f)
        wm[f"c0w{k}"] = np.ascontiguousarray(c0_w[k].T).astype(f)
    wm["h0b"] = np.ascontiguousarray(h0_b.T).astype(f)
    wm["c0b"] = np.ascontiguousarray(c0_b.T).astype(f)
    p1T = np.ascontiguousarray(p1_w.T).astype(f)  # [354,128]
    wm["p1wA"] = p1T[0:128]
    wm["p1wB"] = p1T[128:256]
    wm["p1wC"] = np.ascontiguousarray(p1T[256:354])
    wm["p1b"] = p1_b.reshape(-1, 1).astype(f)
    wm["p2w"] = np.ascontiguousarray(p2_w.T).astype(f)
    wm["p2b"] = p2_b.reshape(-1, 1).astype(f)
    wm["p3w"] = np.ascontiguousarray(p3_w.T).astype(f)
    wm["p3b"] = p3_b.reshape(-1, 1).astype(f)
    wm["idF"] = np.eye(128, dtype=f16)
    wm["idS"] = np.eye(64, dtype=f)
    msk = np.zeros((128, 128), f16)
    for p in range(128):
        for hh in range(4):
            msk[p, hh * 32 + p % 32] = 1.0
    wm["mask"] = msk
    return wm


def _run_device(x_tag, x_com_first, x_com_last, wm, debug=False):
    from concourse import bass_utils
    nc = _get_nc(debug)
    x16 = x_tag.astype(np.float16)
    xcfT = x_com_first.astype(np.float32).T  # [49, 256]
    xclT = x_com_last.astype(np.float32).T
    in_maps = []
    for c in range(N_CORES):
        m = dict(wm)
        m["x"] = x16[c * BC:(c + 1) * BC]
        m["xcf"] = np.ascontiguousarray(xcfT[:, c * BC:(c + 1) * BC])
        m["xcl"] = np.ascontiguousarray(xclT[:, c * BC:(c + 1) * BC])
        in_maps.append(m)
    res = bass_utils.run_bass_kernel_spmd(nc, in_maps, core_ids=list(range(N_CORES)))
    out = np.empty((B, 16), np.float32)
    for c in range(N_CORES):
        out[c * BC:(c + 1) * BC] = res.results[c]["y"].T
    if debug:
        return out, res
    return out


def kernel(x_tag, x_com_first, x_com_last, pre_w, pre_b, h0_w, h0_b, c0_w, c0_b,
           rnn0_wih, rnn1_wih, rnn_whh, rnn_bih, rnn_bhh,
           adh_w, adh_b, adc_w, adc_b, ar_wih, ar_whh, ar_bih, ar_bhh,
           p1_w, p1_b, p2_w, p2_b, p3_w, p3_b):
    args = dict(pre_w=pre_w, pre_b=pre_b, h0_w=h0_w, h0_b=h0_b, c0_w=c0_w, c0_b=c0_b,
                rnn0_wih=rnn0_wih, rnn1_wih=rnn1_wih, rnn_whh=rnn_whh,
                rnn_bih=rnn_bih, rnn_bhh=rnn_bhh, adh_w=adh_w, adh_b=adh_b,
                adc_w=adc_w, adc_b=adc_b, ar_wih=ar_wih, ar_whh=ar_whh,
                ar_bih=ar_bih, ar_bhh=ar_bhh, p1_w=p1_w, p1_b=p1_b,
                p2_w=p2_w, p2_b=p2_b, p3_w=p3_w, p3_b=p3_b)
    wm = _prep_weights(**{k: np.asarray(v) for k, v in args.items()})
    try:
        return _run_device(np.asarray(x_tag, np.float32),
                           np.asarray(x_com_first, np.float32),
                           np.asarray(x_com_last, np.float32), wm)
    except Exception:
        import traceback
        traceback.print_exc()
        return _kernel_numpy(np.asarray(x_tag, np.float32),
                             np.asarray(x_com_first, np.float32),
                             np.asarray(x_com_last, np.float32),
                             **{k: np.asarray(v, np.float32) for k, v in args.items()})


def _sigmoid(x):
    return 1.0 / (1.0 + np.exp(-x))


def _lstm_np(pre, h, c, whh, bhh, reverse=False):
    Bq, Tq, G = pre.shape
    Hh = G // 4
    whh_T = whh.T.astype(np.float32)
    hs = np.empty((Bq, Tq, Hh), np.float32)
    ts = range(Tq - 1, -1, -1) if reverse else range(Tq)
    for t in ts:
        g = pre[:, t] + h @ whh_T + bhh
        i, f, gg, o = np.split(g, 4, axis=-1)
        c = _sigmoid(f) * c + _sigmoid(i) * np.tanh(gg)
        h = _sigmoid(o) * np.tanh(c)
        hs[:, t] = h
    return hs, h, c


def _kernel_numpy(x_tag, x_com_first, x_com_last, pre_w, pre_b, h0_w, h0_b, c0_w, c0_b,
                  rnn0_wih, rnn1_wih, rnn_whh, rnn_bih, rnn_bhh,
                  adh_w, adh_b, adc_w, adc_b, ar_wih, ar_whh, ar_bih, ar_bhh,
                  p1_w, p1_b, p2_w, p2_b, p3_w, p3_b):
    f = np.float32
    xc = np.stack([x_com_first, x_com_last, x_com_first, x_com_last]).astype(f)
    h0 = np.einsum('kbd,khd->kbh', xc, h0_w).astype(f) + h0_b[:, None, :]
    c0 = np.einsum('kbd,khd->kbh', xc, c0_w).astype(f) + c0_b[:, None, :]
    h_tag = (x_tag.reshape(-1, DIM_TAG) @ pre_w.T).reshape(B, T, H) + pre_b

    def inproj(xseq, wih, bih):
        r = xseq.reshape(B * T, -1) @ wih.T.astype(f) + bih
        return r.reshape(B, T, -1).astype(f)

    hf0, _, _ = _lstm_np(inproj(h_tag, rnn0_wih[0], rnn_bih[0, 0]), h0[0], c0[0], rnn_whh[0, 0], rnn_bhh[0, 0], False)
    hb0, _, _ = _lstm_np(inproj(h_tag, rnn0_wih[1], rnn_bih[0, 1]), h0[1], c0[1], rnn_whh[0, 1], rnn_bhh[0, 1], True)
    x1 = np.concatenate([hf0, hb0], axis=-1)
    hf1, hnf, cnf = _lstm_np(inproj(x1, rnn1_wih[0], rnn_bih[1, 0]), h0[2], c0[2], rnn_whh[1, 0], rnn_bhh[1, 0], False)
    hb1, hnb, cnb = _lstm_np(inproj(x1, rnn1_wih[1], rnn_bih[1, 1]), h0[3], c0[3], rnn_whh[1, 1], rnn_bhh[1, 1], True)
    h_out = np.concatenate([hf1, hb1], axis=-1)
    hn_sel = np.stack([hnb, hnb, hnf, hnf])
    cn_sel = np.stack([cnb, cnb, cnf, cnf])
    h0a = (np.einsum('kbd,kd->kb', hn_sel, adh_w).astype(f) + adh_b[:, None])[..., None]
    c0a = (np.einsum('kbd,kd->kb', cn_sel, adc_w).astype(f) + adc_b[:, None])[..., None]
    attn = np.empty((4, B, T, 1), f)
    for k in range(4):
        attn[k] = _lstm_np(inproj(h_out, ar_wih[k], ar_bih[k]), h0a[k], c0a[k], ar_whh[k], ar_bhh[k], False)[0]
    attn = np.exp(attn - attn.max(axis=2, keepdims=True))
    attn = attn / attn.sum(axis=2, keepdims=True)
    pooled = np.sum(attn * h_out[None], axis=2)
    hh = np.concatenate([pooled[0], pooled[1], pooled[2], pooled[3], x_com_first, x_com_last], axis=1).astype(f)
    hh = np.maximum(hh @ p1_w.T.astype(f) + p1_b, 0.0)
    hh = np.maximum(hh @ p2_w.T.astype(f) + p2_b, 0.0)
    return _sigmoid(hh @ p3_w.T.astype(f) + p3_b).astype(f)
